# revision 1
# baseline (speedup 1.0000x reference)
"""Trainium2 Bass kernel for nn_DecLayer (GNN message-passing decoder layer).

Strategy
--------
Data-parallel over nodes: 10000 nodes are padded and split 1280 per core
across 8 NeuronCores.  Per core everything is computed in "transposed space"
(feature dim on SBUF partitions, edges/nodes on the free dim), which makes
every matmul a clean contraction with the weights as the stationary operand
and needs no on-chip transposes: the host hands the kernel h_E / h_V already
feature-major (same bytes, different layout).

Per node-group (GROUP nodes, supertiles of 512 edges):
  h1.T = W1e.T @ hE.T + W1v.T @ bcast(hV.T)       (PSUM accumulate, bf16)
  g1 = gelu(h1.T + b1)                            (ScalarE, bias per-partition)
  h2.T = W2.T @ g1; g2 = gelu(h2.T + b2)
  masked neighbor sum over K=32 on the free dim   (VectorE [mult +] reduce)
then per group: W3 matmul + rank-1 (b3 x masksum), residual, LayerNorm with
cross-partition stats via ones-matmuls (float32r), FFN (Win/Wout), LayerNorm2,
mask_V.  Output is produced feature-major and transposed back on the host.

If the runtime mask arrays are all ones (true for this problem's
setup_inputs), the kernel is compiled without the mask-broadcast matmul and
mask multiplies; the general path handles arbitrary masks.
"""

import json

import numpy as np
import ml_dtypes

import concourse.bass as bass
import concourse.mybir as mybir
import concourse.tile as tile

F32 = mybir.dt.float32
F32R = mybir.dt.float32r
BF16 = mybir.dt.bfloat16
AF = mybir.ActivationFunctionType
OP = mybir.AluOpType
AX = mybir.AxisListType

H = 128        # hidden
C = 384        # edge feature dim
K = 32         # neighbors
N_CORES = 8
GROUP = 128                      # nodes per group
ST_NODES = 16                    # nodes per supertile
ST_E = ST_NODES * K              # 512 edges per supertile
SCALE = 30.0
EPS = 1e-5


# ---------------------------------------------------------------------------
# walrus workaround: this build rejects >1 sync wait per instruction; split
# extra waits into standalone EventSemaphore instructions on the same engine
# (engines execute their stream in order, so semantics are preserved).
# ---------------------------------------------------------------------------
def _split_multi_waits(bir_json: bytes) -> bytes:
    m = json.loads(bir_json)
    for f in m.get("functions", []):
        for b in f.get("blocks", []):
            out = []
            for inst in b.get("instructions", []):
                si = inst.get("sync_info")
                waits = (si or {}).get("on_wait") or []
                if len(waits) > 1:
                    for j, w in enumerate(waits[:-1]):
                        out.append({
                            "debug": inst.get("debug", 0),
                            "engine": inst["engine"],
                            "ins": [], "outs": [],
                            "name": f"{inst['name']}_wsplit{j}",
                            "opcode": "EventSemaphore",
                            "sync_info": {"on_update": [], "on_wait": [w]},
                        })
                    si["on_wait"] = waits[-1:]
                out.append(inst)
            b["instructions"] = out
    return json.dumps(m).encode()


def _install_bir_fix():
    import concourse.bass_utils as bu
    import concourse.bass2jax as b2j
    if getattr(bu, "_wsplit_installed", False):
        return
    orig = bu.compile_bir_kernel

    def patched(bir_json, tmpdir, neff_name="file.neff"):
        return orig(_split_multi_waits(bir_json), tmpdir, neff_name)

    bu.compile_bir_kernel = patched
    b2j.compile_bir_kernel = patched
    bu._wsplit_installed = True


def _bf(x):
    return np.asarray(x, dtype=ml_dtypes.bfloat16)


def _r(ap):
    """fp32 matmul operand passthrough.  (float32r would double PE throughput
    on these small stats/broadcast matmuls, but this walrus build's verifier
    requires explicitly-rounded fp32r producers, so plain fp32 is used.)"""
    return ap


def build_nc(w, n_nodes, ones_masks=False):
    """Build the per-core Bass kernel. `w` holds the (host numpy) weights.

    n_nodes must be a multiple of GROUP.
    """
    assert n_nodes % GROUP == 0
    n_groups = n_nodes // GROUP
    n_edges = n_nodes * K

    nc = bass.Bass()

    hEt = nc.dram_tensor("hEt", [C, n_edges], F32, kind="ExternalInput")
    hVt = nc.dram_tensor("hVt", [H, n_nodes], F32, kind="ExternalInput")
    maskr = nc.dram_tensor("maskr", [1, n_edges], F32, kind="ExternalInput")
    msumb = nc.dram_tensor("msumb", [1, n_nodes], BF16, kind="ExternalInput")
    maskVr = nc.dram_tensor("maskVr", [1, n_nodes], F32, kind="ExternalInput")
    outt = nc.dram_tensor("outt", [H, n_nodes], F32, kind="ExternalOutput")

    # --- inline constants (weights are known at build time) ---
    W1 = w["W1_w"]
    w1e_h = np.concatenate([W1[H + 128 * j: H + 128 * (j + 1), :] for j in range(3)], axis=1)
    wout_h = np.concatenate([w["Wout_w"][128 * j: 128 * (j + 1), :] for j in range(4)], axis=1)
    w1e = nc.inline_tensor(_bf(w1e_h), name="w1e")
    w1v = nc.inline_tensor(_bf(W1[:H, :]), name="w1v")
    w2 = nc.inline_tensor(_bf(w["W2_w"]), name="w2")
    w3s = nc.inline_tensor(_bf(w["W3_w"] / SCALE), name="w3s")
    b3s_row = nc.inline_tensor(_bf(w["W3_b"] / SCALE).reshape(1, H), name="b3s")
    win = nc.inline_tensor(_bf(w["Win_w"]), name="win")
    wout = nc.inline_tensor(_bf(wout_h), name="wout")
    woutb_row = nc.inline_tensor(_bf(w["Wout_b"]).reshape(1, H), name="woutb")
    ones_row_b = nc.inline_tensor(np.ones((1, GROUP), ml_dtypes.bfloat16), name="onesrb")
    ones_row_f = nc.inline_tensor(np.ones((1, H), np.float32), name="onesrf")
    ones_col_f = nc.inline_tensor(np.ones((H, 1), np.float32), name="onescf")
    # bcast-matmul lhsT rows with folded constants:
    #   invH_row: mean bcast from raw column-sum (x 1/H)
    #   g1r/g2r: rstd bcast with LayerNorm gamma folded in (per-partition)
    invH_row = nc.inline_tensor(np.full((1, H), 1.0 / H, np.float32), name="invhr")
    g1_row = nc.inline_tensor(w["ln1_g"].astype(np.float32).reshape(1, H), name="g1row")
    g2_row = nc.inline_tensor(w["ln2_g"].astype(np.float32).reshape(1, H), name="g2row")
    # per-partition columns: b1, b2, ln1g, ln1b, ln2g, ln2b, winb0..3, eps
    cols_h = np.stack(
        [w["W1_b"], w["W2_b"], w["ln1_g"], w["ln1_b"], w["ln2_g"], w["ln2_b"]]
        + [w["Win_b"][128 * j: 128 * (j + 1)] for j in range(4)]
        + [np.full(H, EPS, np.float32)],
        axis=1,
    ).astype(np.float32)  # [128, 11]
    colsD = nc.inline_tensor(cols_h, name="cols")

    n_st = GROUP // ST_NODES

    # value specializations (checked against the actual weights at build time)
    b1z = not np.any(w["W1_b"])
    b2z = not np.any(w["W2_b"])
    b3z = not np.any(w["W3_b"])
    winbz = not np.any(w["Win_b"])
    woutbz = not np.any(w["Wout_b"])
    ln1bz = not np.any(w["ln1_b"])
    ln2bz = not np.any(w["ln2_b"])

    with tile.TileContext(nc) as tc:
        with (
            tc.tile_pool(name="const", bufs=1) as constp,
            tc.tile_pool(name="xe", bufs=2) as xep,
            tc.tile_pool(name="st", bufs=3) as stp,
            tc.tile_pool(name="grp", bufs=3) as grpp,
            tc.tile_pool(name="ps_st", bufs=2, space="PSUM") as pst,
            tc.tile_pool(name="ps_grp", bufs=2, space="PSUM") as pgr,
        ):
            # load constants once
            w1e_s = constp.tile([H, 3 * 128], BF16)
            nc.sync.dma_start(w1e_s[:], w1e[:])
            w1v_s = constp.tile([H, H], BF16)
            nc.sync.dma_start(w1v_s[:], w1v[:])
            w2_s = constp.tile([H, H], BF16)
            nc.sync.dma_start(w2_s[:], w2[:])
            w3s_s = constp.tile([H, H], BF16)
            nc.sync.dma_start(w3s_s[:], w3s[:])
            b3s_s = constp.tile([1, H], BF16)
            nc.sync.dma_start(b3s_s[:], b3s_row[:])
            win_s = constp.tile([H, 512], BF16)
            nc.sync.dma_start(win_s[:], win[:])
            wout_s = constp.tile([H, 512], BF16)
            nc.sync.dma_start(wout_s[:], wout[:])
            woutb_s = constp.tile([1, H], BF16)
            nc.sync.dma_start(woutb_s[:], woutb_row[:])
            onesrb_s = constp.tile([1, GROUP], BF16)
            nc.sync.dma_start(onesrb_s[:], ones_row_b[:])
            onesrf_s = constp.tile([1, H], F32)
            nc.sync.dma_start(onesrf_s[:], ones_row_f[:])
            onescf_s = constp.tile([H, 1], F32)
            nc.sync.dma_start(onescf_s[:], ones_col_f[:])
            invh_s = constp.tile([1, H], F32)
            nc.sync.dma_start(invh_s[:], invH_row[:])
            g1r_s = constp.tile([1, H], F32)
            nc.sync.dma_start(g1r_s[:], g1_row[:])
            g2r_s = constp.tile([1, H], F32)
            nc.sync.dma_start(g2r_s[:], g2_row[:])
            cols_s = constp.tile([H, 11], F32)
            nc.sync.dma_start(cols_s[:], colsD[:])

            def col(i):
                return cols_s[:, i:i + 1]

            def ln_stats(x):
                """Cross-partition mean/rstd of x [128, GROUP] via ones-matmuls."""
                sq = grpp.tile([H, GROUP], F32, tag="sq")
                nc.vector.tensor_tensor(sq[:], x[:], x[:], op=OP.mult)
                ps1 = pgr.tile([1, GROUP], F32, tag="gp")
                nc.tensor.matmul(ps1[:], _r(onescf_s[:]), _r(x[:]))
                ps2 = pgr.tile([1, GROUP], F32, tag="gp")
                nc.tensor.matmul(ps2[:], _r(onescf_s[:]), _r(sq[:]))
                mean = grpp.tile([1, GROUP], F32, tag="mean")
                nc.vector.tensor_scalar_mul(mean[:], ps1[:], 1.0 / H)
                m2 = grpp.tile([1, GROUP], F32, tag="m2")
                nc.vector.tensor_tensor(m2[:], mean[:], mean[:], op=OP.mult)
                var = grpp.tile([1, GROUP], F32, tag="var")
                # var = ps2/H - mean^2 in one fused op
                nc.vector.scalar_tensor_tensor(
                    var[:], ps2[:], 1.0 / H, m2[:],
                    op0=OP.mult, op1=OP.subtract,
                )
                sd = grpp.tile([1, GROUP], F32, tag="sd")
                nc.scalar.activation(sd[:], var[:], AF.Sqrt, bias=cols_s[0:1, 10:11])
                rstd = grpp.tile([1, GROUP], F32, tag="rstd")
                nc.vector.reciprocal(rstd[:], sd[:])
                return {"mean": mean, "rstd": rstd}

            def ln_finish(x, stats, g_i, b_i, extra_mul=None):
                # pmb = ones x mean ; prb = gamma x rstd (gamma folded in lhsT)
                pmb = pgr.tile([H, GROUP], F32, tag="gp")
                nc.tensor.matmul(pmb[:], _r(onesrf_s[:]), _r(stats["mean"][:]))
                prb = pgr.tile([H, GROUP], F32, tag="gp")
                nc.tensor.matmul(
                    prb[:], _r(g1r_s[:] if g_i == 2 else g2r_s[:]),
                    _r(stats["rstd"][:]),
                )
                t1 = grpp.tile([H, GROUP], F32, tag="t1")
                nc.vector.tensor_tensor(t1[:], x[:], pmb[:], op=OP.subtract)
                t2 = grpp.tile([H, GROUP], F32, tag="t2")
                nc.vector.tensor_tensor(t2[:], t1[:], prb[:], op=OP.mult)
                beta_zero = ln1bz if g_i == 2 else ln2bz
                if beta_zero:
                    xn = t2
                else:
                    xn = grpp.tile([H, GROUP], F32, tag="xn")
                    nc.vector.tensor_scalar(
                        xn[:], t2[:], scalar1=col(b_i), scalar2=None, op0=OP.add,
                    )
                if extra_mul is not None:
                    xm = grpp.tile([H, GROUP], F32, tag="xm")
                    nc.vector.tensor_tensor(xm[:], xn[:], extra_mul[:], op=OP.mult)
                    return xm
                return xn

            # ---------------- software-pipelined emission ----------------
            # Engines execute their instruction streams in order, so emission
            # order shapes the pipeline.  Per flat supertile index t we emit:
            #   A(t): W1 matmuls into ph1(t)          [PE]
            #   C(t-1): W2 matmul into ph2(t-1)       [PE]
            #   D(t-1): gelu2 (+mask mult)            [ACT/DVE]
            #   B(t): gelu1                           [ACT]
            #   E(t-1): neighbor-sum reduce           [DVE]
            # so PE never waits on the same supertile's activations.  Group
            # tails (W3/LN/FFN/LN2/store) are split into parts emitted one
            # per iteration while the next group's supertiles stream.
            gstate = {}   # g -> dict(xe, hv_f, hv_b, msum, mV, agg)
            ststate = {}  # t -> dict(ph1, ph2, g1, g2, pm)

            def group_loads(g):
                e0 = g * GROUP * K
                n0 = g * GROUP
                st = {}
                # small per-group loads first so they land before the bulk
                # h_E stream saturates the DMA engines
                st["hv_f"] = grpp.tile([H, GROUP], F32, tag="hv_f", name="hv_f")
                nc.sync.dma_start(st["hv_f"][:], hVt[:, n0:n0 + GROUP])
                st["hv_b"] = grpp.tile([H, GROUP], BF16, tag="hv_b", name="hv_b")
                nc.vector.tensor_copy(st["hv_b"][:], st["hv_f"][:])
                st["msum"] = grpp.tile([1, GROUP], BF16, tag="msum", name="msum")
                nc.sync.dma_start(st["msum"][:], msumb[:, n0:n0 + GROUP])
                st["xe"] = xep.tile([H, 3, GROUP * K], BF16, tag="xe", name="xe")
                # six half-chunk cast-DMAs per group: finer issue granularity
                # keeps the SWDGE stream busy and lets compute start earlier.
                # hh-major order: the first three transfers cover the first
                # half of every c-chunk, so supertile 0 starts after 3 of 6.
                half = GROUP * K // 2
                for hh in range(2):
                    for j in range(3):
                        nc.gpsimd.dma_start(
                            st["xe"][:, j, hh * half:(hh + 1) * half],
                            hEt[128 * j:128 * (j + 1),
                                e0 + hh * half:e0 + (hh + 1) * half],
                        )
                if not ones_masks:
                    st["mV"] = grpp.tile([1, GROUP], F32, tag="mV", name="mV")
                    nc.sync.dma_start(st["mV"][:], maskVr[:, n0:n0 + GROUP])
                st["agg"] = grpp.tile([H, GROUP], BF16, tag="agg", name="agg")
                gstate[g] = st

            mstate = {}

            def mrow_load(t):
                if ones_masks or t >= n_groups * n_st:
                    return
                mrow = stp.tile([1, ST_E], F32, tag="mrow")
                nc.sync.dma_start(mrow[:], maskr[:, t * ST_E:(t + 1) * ST_E])
                mstate[t] = mrow

            def stage_A(t):
                g, s = divmod(t, n_st)
                gs = gstate[g]
                c0 = s * ST_E
                st = {}
                ph1 = pst.tile([H, ST_E], F32, tag="ph1")
                for j in range(3):
                    nc.tensor.matmul(
                        ph1[:], w1e_s[:, 128 * j:128 * (j + 1)],
                        gs["xe"][:, j, c0:c0 + ST_E],
                        start=(j == 0), stop=False,
                    )
                hvs = gs["hv_b"][:, s * ST_NODES:(s + 1) * ST_NODES]
                nc.tensor.matmul(
                    ph1[:], w1v_s[:], hvs.broadcast_to([H, ST_NODES, K]),
                    start=False, stop=True,
                )
                st["ph1"] = ph1
                if not ones_masks:
                    pm = pst.tile([H, ST_E], F32, tag="pm")
                    nc.tensor.matmul(pm[:], _r(onesrf_s[:]), _r(mstate.pop(t)[:]))
                    st["pm"] = pm
                ststate[t] = st

            def stage_B(t):
                st = ststate[t]
                g1 = stp.tile([H, ST_E], BF16, tag="g1")
                nc.scalar.activation(g1[:], st["ph1"][:], AF.Gelu,
                                      bias=0.0 if b1z else col(0))
                st["g1"] = g1

            def stage_C(t):
                st = ststate[t]
                ph2 = pst.tile([H, ST_E], F32, tag="ph2")
                nc.tensor.matmul(ph2[:], w2_s[:], st["g1"][:])
                st["ph2"] = ph2

            def stage_D(t):
                st = ststate[t]
                g2 = stp.tile([H, ST_E], BF16, tag="g2")
                nc.scalar.activation(g2[:], st["ph2"][:], AF.Gelu,
                                      bias=0.0 if b2z else col(1))
                if ones_masks:
                    st["red"] = g2
                else:
                    g2m = stp.tile([H, ST_E], BF16, tag="g2m")
                    nc.vector.tensor_tensor(g2m[:], g2[:], st["pm"][:], op=OP.mult)
                    st["red"] = g2m

            def stage_E(t):
                g, s = divmod(t, n_st)
                st = ststate.pop(t)
                with nc.allow_low_precision("32-term neighbor sum fits bf16"):
                    nc.vector.reduce_sum(
                        gstate[g]["agg"][:, s * ST_NODES:(s + 1) * ST_NODES],
                        st["red"].rearrange("p (n k) -> p n k", k=K),
                        axis=AX.X,
                    )

            def tail_parts(g):
                gs = gstate[g]
                n0 = g * GROUP
                ctx = {}

                def p1():
                    pdh = pgr.tile([H, GROUP], F32, tag="gp")
                    nc.tensor.matmul(pdh[:], w3s_s[:], gs["agg"][:],
                                     start=True, stop=b3z)
                    if not b3z:
                        nc.tensor.matmul(pdh[:], b3s_s[:], gs["msum"][:],
                                         start=False, stop=True)
                    x = grpp.tile([H, GROUP], F32, tag="x")
                    nc.vector.tensor_tensor(x[:], gs["hv_f"][:], pdh[:], op=OP.add)
                    ctx["x"] = x

                def p2():
                    ctx["ln1"] = ln_stats(ctx["x"])

                def p3():
                    xln = ln_finish(ctx["x"], ctx["ln1"], 2, 3)
                    xlnb = grpp.tile([H, GROUP], BF16, tag="xlnb")
                    nc.vector.tensor_copy(xlnb[:], xln[:])
                    ctx["xln"], ctx["xlnb"] = xln, xlnb

                def p4():
                    gf = grpp.tile([H, 4, GROUP], BF16, tag="gf")
                    if winbz:
                        # zero bias: one [128, 4*GROUP] gelu over a single
                        # PSUM bank instead of four, amortizing ACT overhead
                        pf = pgr.tile([H, 4, GROUP], F32, tag="gp", name="pf")
                        for j in range(4):
                            nc.tensor.matmul(
                                pf[:, j, :], win_s[:, 128 * j:128 * (j + 1)],
                                ctx["xlnb"][:],
                            )
                        nc.scalar.activation(gf[:], pf[:], AF.Gelu, bias=0.0)
                    else:
                        for j in range(4):
                            pf = pgr.tile([H, GROUP], F32, tag="gp", name="pf")
                            nc.tensor.matmul(
                                pf[:], win_s[:, 128 * j:128 * (j + 1)],
                                ctx["xlnb"][:],
                            )
                            nc.scalar.activation(gf[:, j, :], pf[:], AF.Gelu,
                                                 bias=col(6 + j))
                    ctx["gf"] = gf

                def p5():
                    py = pgr.tile([H, GROUP], F32, tag="gp")
                    for j in range(4):
                        nc.tensor.matmul(
                            py[:], wout_s[:, 128 * j:128 * (j + 1)],
                            ctx["gf"][:, j, :], start=(j == 0),
                            stop=(woutbz and j == 3),
                        )
                    if not woutbz:
                        nc.tensor.matmul(py[:], woutb_s[:], onesrb_s[:],
                                         start=False, stop=True)
                    z = grpp.tile([H, GROUP], F32, tag="z")
                    nc.vector.tensor_tensor(z[:], ctx["xln"][:], py[:], op=OP.add)
                    ctx["z"] = z

                def p6():
                    ctx["ln2"] = ln_stats(ctx["z"])

                def p7():
                    if ones_masks:
                        outf = ln_finish(ctx["z"], ctx["ln2"], 4, 5)
                    else:
                        pmv = pgr.tile([H, GROUP], F32, tag="gp")
                        nc.tensor.matmul(pmv[:], _r(onesrf_s[:]), _r(gs["mV"][:]))
                        outf = ln_finish(ctx["z"], ctx["ln2"], 4, 5, extra_mul=pmv)
                    nc.sync.dma_start(outt[:, n0:n0 + GROUP], outf[:])
                    gstate.pop(g)

                return [p1, p2, p3, p4, p5, p6, p7]

            pending = []
            total = n_groups * n_st
            group_loads(0)
            mrow_load(0)
            mrow_load(1)
            for t in range(total + 1):
                g, s = divmod(t, n_st)
                if t < total:
                    if s == 1 and g + 1 < n_groups:
                        group_loads(g + 1)
                    mrow_load(t + 2)
                    stage_A(t)
                if t >= 1:
                    stage_C(t - 1)
                    stage_D(t - 1)
                if t < total:
                    stage_B(t)
                if t >= 1:
                    stage_E(t - 1)
                    if s == 0:
                        pending.extend(tail_parts(g - 1))
                if pending:
                    pending.pop(0)()
            while pending:
                pending.pop(0)()

    return nc


def _prep_core_inputs(h_V, h_E, mask_V, mask_attend, n_pad):
    """Host marshalling for one core's node slice (feature-major layouts)."""
    n = h_V.shape[0]
    hEt = np.zeros((C, n_pad * K), np.float32)
    hEt[:, : n * K] = h_E.reshape(n * K, C).T
    hVt = np.zeros((H, n_pad), np.float32)
    hVt[:, :n] = h_V.T
    maskr = np.zeros((1, n_pad * K), np.float32)
    maskr[:, : n * K] = mask_attend.reshape(1, n * K)
    msumb = np.zeros((1, n_pad), ml_dtypes.bfloat16)
    msumb[:, :n] = _bf(mask_attend.sum(axis=1, dtype=np.float32)).reshape(1, n)
    maskVr = np.zeros((1, n_pad), np.float32)
    maskVr[:, :n] = mask_V.reshape(1, n)
    return {
        "hEt": np.ascontiguousarray(hEt),
        "hVt": np.ascontiguousarray(hVt),
        "maskr": maskr,
        "msumb": msumb,
        "maskVr": maskVr,
    }


def kernel(h_V, h_E, mask_V, mask_attend,
           W1_w, W1_b, W2_w, W2_b, W3_w, W3_b,
           ln1_g, ln1_b, Win_w, Win_b, Wout_w, Wout_b, ln2_g, ln2_b):
    from concourse.bass_utils import run_bass_kernel_spmd

    _install_bir_fix()

    h_V = np.asarray(h_V, np.float32)
    h_E = np.asarray(h_E, np.float32)
    mask_V = np.asarray(mask_V, np.float32)
    mask_attend = np.asarray(mask_attend, np.float32)

    n_full = h_V.shape[0]
    per = (n_full + N_CORES - 1) // N_CORES          # 1250
    n_pad = ((per + GROUP - 1) // GROUP) * GROUP     # 1280

    w = dict(W1_w=W1_w, W1_b=W1_b, W2_w=W2_w, W2_b=W2_b, W3_w=W3_w, W3_b=W3_b,
             ln1_g=ln1_g, ln1_b=ln1_b, Win_w=Win_w, Win_b=Win_b,
             Wout_w=Wout_w, Wout_b=Wout_b, ln2_g=ln2_g, ln2_b=ln2_b)
    w = {k: np.asarray(v, np.float32) for k, v in w.items()}

    ones_masks = bool(np.all(mask_attend == 1.0) and np.all(mask_V == 1.0))
    nc = build_nc(w, n_pad, ones_masks=ones_masks)

    in_maps = []
    for c in range(N_CORES):
        lo, hi = c * per, min((c + 1) * per, n_full)
        in_maps.append(_prep_core_inputs(
            h_V[lo:hi], h_E[lo:hi], mask_V[lo:hi], mask_attend[lo:hi], n_pad
        ))

    res = run_bass_kernel_spmd(nc, in_maps, core_ids=list(range(N_CORES)))

    out = np.empty((n_full, H), np.float32)
    for c in range(N_CORES):
        lo, hi = c * per, min((c + 1) * per, n_full)
        out[lo:hi] = res.results[c]["outt"].T[: hi - lo]
    return out



# revision 55
# speedup vs baseline: 1.5073x; 1.5073x over previous
"""Trainium2 Bass kernel for nn_DecLayer (GNN message-passing decoder layer).

Strategy
--------
Data-parallel over nodes: 10000 nodes are padded and split 1280 per core
across 8 NeuronCores.  Per core everything is computed in "transposed space"
(feature dim on SBUF partitions, edges/nodes on the free dim).

Key optimizations over the v1 kernel:
  * h_E is pre-quantized to fp8(e4m3) on the host -> HBM traffic for the
    dominant stream drops 4x vs f32 (15.7MB/core, ~44us at modeled BW).
  * All edge-level matmuls run in fp8 DoubleRow perf mode (2 contraction
    tiles per pass, 0.5 cycles/row).  Weights are pre-scaled by powers of
    two into the fp8 normal range; the activation instruction's `scale`
    operand undoes the scaling for free.  Odd contraction tiles are padded
    by reading a data tile twice against zero weights (stride-0 AP).
  * The K=32 neighbor sum is folded into the W3 matmul: 32 accumulating
    PSUM matmuls with k-strided moving APs replace the (slow, no-perf-mode)
    DVE TensorReduce entirely.
  * LayerNorm stats/broadcasts use bf16 ones-matmuls (1 cycle/row instead
    of 4 for f32).
  * gelu instructions cover 1024 edges (2 PSUM banks) to amortize ACT
    SBUF-access overhead; ScalarE is the bottleneck engine at ~93us busy.
    Individual matmuls stay within one PSUM bank (512 f32) as the ISA
    requires; the 2-bank activation reads span both halves.
  * Group tails (W3/LN/FFN/LN2/store) are split into parts and interleaved
    into the supertile pipeline; the last group's tail runs as two
    half-width chains, the first overlapping the final edge supertiles.
  * h_E arrives via one merged 3-feature-chunk SWDGE descriptor per
    half-group (group 0 in progressively larger chunks so compute starts
    ~5us after launch).

Modeled per-core time: ~119us (vs 180us for the f32/bf16 v1 kernel);
measured end-to-end hardware rel-err vs the f32 reference: 5.8e-3.

If the runtime masks are all ones (true for this problem's setup_inputs),
the kernel compiles without any mask handling; a general path handles
arbitrary masks.
"""

import json

import numpy as np
import ml_dtypes

import concourse.bass as bass
import concourse.mybir as mybir
import concourse.tile as tile

F32 = mybir.dt.float32
BF16 = mybir.dt.bfloat16
FP8 = mybir.dt.float8e4
AF = mybir.ActivationFunctionType
OP = mybir.AluOpType
AX = mybir.AxisListType
PM = mybir.MatmulPerfMode

E4 = ml_dtypes.float8_e4m3

H = 128        # hidden
C = 384        # edge feature dim
K = 32         # neighbors
N_CORES = 8
GROUP = 256                      # nodes per group
ST_NODES = 32                    # nodes per supertile
ST_E = ST_NODES * K              # 1024 edges per supertile
SCALE = 30.0
EPS = 1e-5
S1 = 16.0                        # W1 fp8 pre-scale
S2 = 8.0                         # W2 fp8 pre-scale
S3 = 256.0 / SCALE               # W3 fp8 pre-scale (folds 1/SCALE)


# ---------------------------------------------------------------------------
# walrus workaround: this build rejects >1 sync wait per instruction; split
# extra waits into standalone EventSemaphore instructions on the same engine
# (engines execute their stream in order, so semantics are preserved).
# ---------------------------------------------------------------------------
def _split_multi_waits(bir_json: bytes) -> bytes:
    m = json.loads(bir_json)
    for f in m.get("functions", []):
        for b in f.get("blocks", []):
            out = []
            for inst in b.get("instructions", []):
                si = inst.get("sync_info")
                waits = (si or {}).get("on_wait") or []
                if len(waits) > 1:
                    for j, w in enumerate(waits[:-1]):
                        out.append({
                            "debug": inst.get("debug", 0),
                            "engine": inst["engine"],
                            "ins": [], "outs": [],
                            "name": f"{inst['name']}_wsplit{j}",
                            "opcode": "EventSemaphore",
                            "sync_info": {"on_update": [], "on_wait": [w]},
                        })
                    si["on_wait"] = waits[-1:]
                out.append(inst)
            b["instructions"] = out
    return json.dumps(m).encode()


def _install_bir_fix():
    import concourse.bass_utils as bu
    import concourse.bass2jax as b2j
    if getattr(bu, "_wsplit_installed", False):
        return
    orig = bu.compile_bir_kernel

    def patched(bir_json, tmpdir, neff_name="file.neff"):
        return orig(_split_multi_waits(bir_json), tmpdir, neff_name)

    bu.compile_bir_kernel = patched
    b2j.compile_bir_kernel = patched
    bu._wsplit_installed = True


def _bf(x):
    return np.asarray(x, dtype=ml_dtypes.bfloat16)


def _f8(x):
    return np.asarray(x, dtype=E4)


def _dr_pair(a, b):
    """Stack two [128, M] weight blocks into DoubleRow lhsT layout
    [128, 2*M] = [p, (t m)]."""
    return np.stack([a, b], axis=1).reshape(128, -1)


def build_nc(w, n_nodes, ones_masks=False):
    """Build the per-core Bass kernel. `w` holds the (host numpy) weights."""
    assert n_nodes % GROUP == 0
    n_groups = n_nodes // GROUP
    n_edges = n_nodes * K
    n_st = GROUP // ST_NODES

    nc = bass.Bass()

    hE8 = nc.dram_tensor("hE8", [C, n_edges], FP8, kind="ExternalInput")
    hv8D = nc.dram_tensor("hv8D", [H, n_nodes], FP8, kind="ExternalInput")
    hVt = nc.dram_tensor("hVt", [H, n_nodes], F32, kind="ExternalInput")
    maskr = nc.dram_tensor("maskr", [1, n_edges], BF16, kind="ExternalInput")
    msumb = nc.dram_tensor("msumb", [1, n_nodes], BF16, kind="ExternalInput")
    maskVr = nc.dram_tensor("maskVr", [1, n_nodes], BF16, kind="ExternalInput")
    outt = nc.dram_tensor("outt", [H, n_nodes], F32, kind="ExternalOutput")

    # --- inline constants (weights known at build time) ---
    W1 = w["W1_w"]          # [H+C, H]
    zero8 = np.zeros((128, H), E4)
    w1a_h = _dr_pair(_f8(W1[H:H + 128] * S1), _f8(W1[H + 128:H + 256] * S1))
    w1b_h = _dr_pair(_f8(W1[H + 256:H + 384] * S1), zero8)
    w1v_h = _dr_pair(_f8(W1[:H] * S1), zero8)
    w2d_h = _dr_pair(_f8(w["W2_w"] * S2), zero8)
    w3d_h = _dr_pair(_f8(w["W3_w"] * S3), zero8)
    w3b_h = _bf(w["W3_w"] / SCALE)                      # general-mask path
    wout_h = np.concatenate(
        [w["Wout_w"][128 * j: 128 * (j + 1), :] for j in range(4)], axis=1)

    w1aD = nc.inline_tensor(w1a_h, name="w1a")
    w1bD = nc.inline_tensor(w1b_h, name="w1b")
    w1vD = nc.inline_tensor(w1v_h, name="w1v")
    w2dD = nc.inline_tensor(w2d_h, name="w2d")
    if ones_masks:
        w3dD = nc.inline_tensor(w3d_h, name="w3d")
    else:
        w3bD = nc.inline_tensor(w3b_h, name="w3b")
        b3sD = nc.inline_tensor(_bf(w["W3_b"] / SCALE).reshape(1, H), name="b3s")
    winD = nc.inline_tensor(_bf(w["Win_w"]), name="win")
    woutD = nc.inline_tensor(_bf(wout_h), name="wout")
    woutbD = nc.inline_tensor(_bf(w["Wout_b"]).reshape(1, H), name="woutb")
    onescD = nc.inline_tensor(np.ones((H, 1), ml_dtypes.bfloat16), name="onesc")
    onesrD = nc.inline_tensor(np.ones((1, H), ml_dtypes.bfloat16), name="onesr")
    onesgD = nc.inline_tensor(np.ones((1, GROUP), ml_dtypes.bfloat16), name="onesg")
    g1rowD = nc.inline_tensor(_bf(w["ln1_g"]).reshape(1, H), name="g1row")
    g2rowD = nc.inline_tensor(_bf(w["ln2_g"]).reshape(1, H), name="g2row")
    # per-partition bias columns: b1, b2, ln1b, ln2b, winb0..3
    cols_h = np.stack(
        [w["W1_b"], w["W2_b"], w["ln1_b"], w["ln2_b"]]
        + [w["Win_b"][128 * j: 128 * (j + 1)] for j in range(4)]
        + [np.full(H, EPS, np.float32)],
        axis=1,
    ).astype(np.float32)
    colsD = nc.inline_tensor(cols_h, name="cols")

    # value specializations (checked against actual weights at build time)
    b1z = not np.any(w["W1_b"])
    b2z = not np.any(w["W2_b"])
    b3z = not np.any(w["W3_b"])
    winbz = not np.any(w["Win_b"])
    woutbz = not np.any(w["Wout_b"])
    ln1bz = not np.any(w["ln1_b"])
    ln2bz = not np.any(w["ln2_b"])

    g2_dt = FP8 if ones_masks else BF16

    with tile.TileContext(nc) as tc:
        with (
            tc.tile_pool(name="const", bufs=1) as constp,
            tc.tile_pool(name="xe", bufs=3) as xep,
            tc.tile_pool(name="g1p", bufs=3) as g1p,
            tc.tile_pool(name="g2p", bufs=3) as g2p,
            tc.tile_pool(name="st", bufs=3) as stp,
            tc.tile_pool(name="grp", bufs=3) as grpp,
            tc.tile_pool(name="ph", bufs=3 if ones_masks else 2,
                         space="PSUM") as php,
            tc.tile_pool(name="pm", bufs=1, space="PSUM") as pmp,
            tc.tile_pool(name="gp", bufs=2, space="PSUM") as pgr,
        ):
            # ---- load constants once (SP/HWDGE; bulk streams go via Pool) ----
            def cload(shape, dt_, src, name):
                t = constp.tile(shape, dt_, name=name)
                nc.sync.dma_start(t[:], src)
                return t

            # hot constants: needed by the first supertile's W1/gelu1/W2
            w1a_s = cload([128, 2, H], FP8, w1aD[:].rearrange("p (t m) -> p t m", t=2), "w1a_s")
            w1b_s = cload([128, 2, H], FP8, w1bD[:].rearrange("p (t m) -> p t m", t=2), "w1b_s")
            w1v_s = cload([128, 2, H], FP8, w1vD[:].rearrange("p (t m) -> p t m", t=2), "w1v_s")
            onesr_s = cload([1, H], BF16, onesrD[:], "onesr_s")

            def cold_consts():
                """Emitted after group-0 loads: first used by W2 / the tails."""
                c = {}
                c["w2d_s"] = cload([128, 2, H], FP8,
                                   w2dD[:].rearrange("p (t m) -> p t m", t=2), "w2d_s")
                c["cols_s"] = cload([H, cols_h.shape[1]], F32, colsD[:], "cols_s")
                if ones_masks:
                    c["w3d_s"] = cload([128, 2, H], FP8,
                                       w3dD[:].rearrange("p (t m) -> p t m", t=2), "w3d_s")
                else:
                    c["w3b_s"] = cload([128, H], BF16, w3bD[:], "w3b_s")
                    c["b3s_s"] = cload([1, H], BF16, b3sD[:], "b3s_s")
                c["win_s"] = cload([H, 512], BF16, winD[:], "win_s")
                c["wout_s"] = cload([H, 512], BF16, woutD[:], "wout_s")
                c["woutb_s"] = cload([1, H], BF16, woutbD[:], "woutb_s")
                c["onesc_s"] = cload([H, 1], BF16, onescD[:], "onesc_s")
                c["onesg_s"] = cload([1, GROUP], BF16, onesgD[:], "onesg_s")
                c["g1row_s"] = cload([1, H], BF16, g1rowD[:], "g1row_s")
                c["g2row_s"] = cload([1, H], BF16, g2rowD[:], "g2row_s")
                return c

            cc = {}

            def col(i):
                return cc["cols_s"][:, i:i + 1]

            # ---------------- per-group state ----------------
            gstate = {}   # g -> dict of tiles
            ststate = {}  # t -> dict of tiles
            mstate = {}

            def group_loads(g):
                e0 = g * GROUP * K
                n0 = g * GROUP
                st = {}
                st["hv8"] = grpp.tile([H, GROUP], FP8, tag="hv8", name="hv8")
                # group 0's hv8 gates the very first W1 matmul: route it via
                # the Pool SWDGE queue, which starts draining before the
                # HWDGE const queue reaches it
                heng = nc.gpsimd if g == 0 else nc.sync
                heng.dma_start(st["hv8"][:], hv8D[:, n0:n0 + GROUP])
                st["hvf"] = grpp.tile([H, GROUP], F32, tag="hvf", name="hvf")
                nc.sync.dma_start(st["hvf"][:], hVt[:, n0:n0 + GROUP])
                if not ones_masks:
                    st["mV"] = grpp.tile([1, GROUP], BF16, tag="mV", name="mV")
                    nc.sync.dma_start(st["mV"][:], maskVr[:, n0:n0 + GROUP])
                    if not b3z:
                        st["msum"] = grpp.tile([1, GROUP], BF16, tag="msum", name="msum")
                        nc.sync.dma_start(st["msum"][:], msumb[:, n0:n0 + GROUP])
                # bulk h_E stream: chunked via gpsimd SWDGE so the HWDGE/SP
                # queue stays free for small loads and stores.  Group 0 uses
                # progressively larger chunks so the first W1 starts early.
                st["xe"] = xep.tile([H, 3, GROUP * K], FP8, tag="xe", name="xe")
                hEr = hE8[:].rearrange("(j p) e -> p j e", p=128)
                if g == 0:
                    bounds = [0, 1024, 2048, 3072, 4096, 6144, 8192]
                else:
                    bounds = [0, 2048, 4096, 8192]
                for hh in range(len(bounds) - 1):
                    nc.gpsimd.dma_start(
                        st["xe"][:, :, bounds[hh]:bounds[hh + 1]],
                        hEr[:, :, e0 + bounds[hh]:e0 + bounds[hh + 1]],
                    )
                st["g2"] = g2p.tile([H, GROUP * K], g2_dt, tag="g2", name="g2")
                gstate[g] = st

            def mrow_load(t):
                if ones_masks or t >= n_groups * n_st:
                    return
                mrow = stp.tile([1, ST_E], BF16, tag="mrow", name="mrow")
                nc.sync.dma_start(mrow[:], maskr[:, t * ST_E:(t + 1) * ST_E])
                mstate[t] = mrow

            def stage_A(t):
                """W1 matmuls for supertile t -> ph (2 PSUM banks)."""
                g, s = divmod(t, n_st)
                gs = gstate[g]
                c0 = s * ST_E
                st = {}
                ph1 = php.tile([H, ST_E], F32, tag="ph", name="ph1")
                HB = ST_E // 2          # matmul out must stay in one PSUM bank
                HN = ST_NODES // 2
                for h in range(2):
                    dst = ph1[:, h * HB:(h + 1) * HB]
                    e0h = c0 + h * HB
                    nc.tensor.matmul(dst, w1a_s[:], gs["xe"][:, 0:2, e0h:e0h + HB],
                                     start=True, stop=False, perf_mode=PM.DoubleRow)
                    nc.tensor.matmul(
                        dst, w1b_s[:],
                        gs["xe"][:, 2:3, e0h:e0h + HB].broadcast_to([H, 2, HB]),
                        start=False, stop=False, perf_mode=PM.DoubleRow)
                    n0h = s * ST_NODES + h * HN
                    hv_b = (gs["hv8"][:, n0h:n0h + HN]
                            .rearrange("p (o n) -> p o n", o=1)
                            .broadcast_to([H, 2, HN])
                            .broadcast_to([H, 2, HN, K]))
                    nc.tensor.matmul(dst, w1v_s[:], hv_b,
                                     start=False, stop=True, perf_mode=PM.DoubleRow)
                st["ph1"] = ph1
                if not ones_masks:
                    pm = pmp.tile([H, ST_E], F32, tag="pm", name="pm")
                    mrow = mstate.pop(t)
                    for h in range(2):
                        nc.tensor.matmul(pm[:, h * (ST_E // 2):(h + 1) * (ST_E // 2)],
                                         onesr_s[:],
                                         mrow[:, h * (ST_E // 2):(h + 1) * (ST_E // 2)])
                    st["pm"] = pm
                ststate[t] = st

            def stage_B(t):
                """gelu1 over 1024 edges; un-scales W1 by 1/S1; fp8 out."""
                st = ststate[t]
                g1 = g1p.tile([H, 1, ST_E], FP8, tag="g1", name="g1")
                nc.scalar.activation(g1[:, 0, :], st["ph1"][:], AF.Gelu,
                                     bias=0.0 if b1z else col(0), scale=1.0 / S1)
                st["g1"] = g1

            def stage_C(t):
                """W2 fp8 DoubleRow (dup-tile) -> ph2."""
                st = ststate[t]
                ph2 = php.tile([H, ST_E], F32, tag="ph", name="ph2")
                HB = ST_E // 2
                for h in range(2):
                    nc.tensor.matmul(
                        ph2[:, h * HB:(h + 1) * HB], cc["w2d_s"][:],
                        st["g1"][:, 0:1, h * HB:(h + 1) * HB]
                        .broadcast_to([H, 2, HB]),
                        start=True, stop=True, perf_mode=PM.DoubleRow)
                st["ph2"] = ph2

            def stage_D(t):
                """gelu2 -> slice of the group g2 tile (fp8 or bf16)."""
                g, s = divmod(t, n_st)
                st = ststate.pop(t)
                gs = gstate[g]
                dst = gs["g2"][:, s * ST_E:(s + 1) * ST_E]
                if ones_masks:
                    nc.scalar.activation(dst, st["ph2"][:], AF.Gelu,
                                         bias=0.0 if b2z else col(1), scale=1.0 / S2)
                else:
                    g2t = stp.tile([H, ST_E], BF16, tag="g2t", name="g2t")
                    nc.scalar.activation(g2t[:], st["ph2"][:], AF.Gelu,
                                         bias=0.0 if b2z else col(1), scale=1.0 / S2)
                    nc.vector.tensor_tensor(dst, g2t[:], st["pm"][:], op=OP.mult)

            # ---------------- group tails ----------------
            def ln_block(src_f32, src_bf, sq_bf, gamma_row, beta_i, beta_z, nm, W,
                         ptile, ve):
                """Shared LN piece: stats matmuls -> mean/rstd -> normalized.

                src_f32: [H, GROUP] f32 (the residual input)
                src_bf/sq_bf: bf16 copies (src, src^2) for the stats matmuls
                Returns list of part-functions; leaves result tile in ctx[nm].
                """
                ctx = {}

                def s1():
                    ps1 = pgr.tile([1, W], F32, tag="gp", name=f"ps1{nm}")
                    nc.tensor.matmul(ps1[:], cc["onesc_s"][:], src_bf())
                    ps2 = pgr.tile([1, W], F32, tag="gp", name=f"ps2{nm}")
                    nc.tensor.matmul(ps2[:], cc["onesc_s"][:], sq_bf())
                    ctx["ps1"], ctx["ps2"] = ps1, ps2

                def s2():
                    mean = grpp.tile([1, W], BF16, tag=f"mean{nm}", name=f"mean{nm}")
                    tmp = grpp.tile([1, W], F32, tag=f"m2{nm}", name=f"m2{nm}")
                    varh = grpp.tile([1, W], F32, tag=f"var{nm}", name=f"var{nm}")
                    sd = grpp.tile([1, W], F32, tag=f"sd{nm}", name=f"sd{nm}")
                    rstd = grpp.tile([1, W], BF16, tag=f"rstd{nm}", name=f"rstd{nm}")
                    with nc.allow_low_precision("bf16 LN stats validated vs ref"):
                        ve.tensor_scalar_mul(mean[:], ctx["ps1"][:], 1.0 / H)
                        ve.tensor_tensor(tmp[:], mean[:], mean[:], op=OP.mult)
                        ve.scalar_tensor_tensor(
                            varh[:], ctx["ps2"][:], 1.0 / H, tmp[:],
                            op0=OP.mult, op1=OP.subtract)
                        nc.scalar.activation(sd[:], varh[:], AF.Sqrt,
                                             bias=cc["cols_s"][0:1, 8:9])
                        nc.vector.reciprocal(rstd[:], sd[:])
                    ctx["mean"], ctx["rstd"] = mean, rstd

                def s3():
                    pmb = ptile([H, W], f"pmb{nm}")
                    nc.tensor.matmul(pmb[:], onesr_s[:], ctx["mean"][:])
                    prb = ptile([H, W], f"prb{nm}")
                    nc.tensor.matmul(prb[:], gamma_row[:], ctx["rstd"][:])
                    ctx["pmb"], ctx["prb"] = pmb, prb

                return ctx, [s1, s2, s3]

            def tail_parts(g, lo=0, W=GROUP, last_piece=True):
                gs = gstate[g]
                n0 = g * GROUP + lo
                nsl = slice(lo, lo + W)
                fin = (g == n_groups - 1) and lo > 0
                ve = nc.vector
                alt = nc.gpsimd
                def ptile(shape, name):
                    if fin:
                        return php.tile(shape, F32, tag="ph", name=name)
                    return pgr.tile(shape, F32, tag="gp", name=name)
                ctx = {}

                def p1(kr=None):  # K-sum folded into W3 (PE)
                    if kr is None:
                        kr = range(K)
                    if "pdh" not in ctx:
                        ctx["pdh"] = pgr.tile([H, W], F32, tag="gp", name="pdh")
                        ctx["g2k"] = (gs["g2"][:]
                                      .rearrange("p (n k) -> p k n", k=K)[:, :, nsl])
                    pdh, g2k = ctx["pdh"], ctx["g2k"]
                    if ones_masks:
                        for k in kr:
                            nc.tensor.matmul(
                                pdh[:], cc["w3d_s"][:],
                                g2k[:, k:k + 1, :].broadcast_to([H, 2, W]),
                                start=(k == 0), stop=(k == K - 1),
                                perf_mode=PM.DoubleRow)
                    else:
                        last = b3z
                        for k in kr:
                            nc.tensor.matmul(
                                pdh[:], cc["w3b_s"][:], g2k[:, k, :],
                                start=(k == 0), stop=(last and k == K - 1))
                        if kr[-1] == K - 1 and not b3z:
                            nc.tensor.matmul(pdh[:], cc["b3s_s"][:],
                                             gs["msum"][:, nsl],
                                             start=False, stop=True)
                    ctx["pdh"] = pdh

                def p2():  # residual add + bf16/sq copies for LN1 stats
                    x = grpp.tile([H, W], F32, tag="x", name="x")
                    pscale = 1.0 / 256.0 if ones_masks else 1.0
                    ve.scalar_tensor_tensor(
                        x[:], ctx["pdh"][:], pscale, gs["hvf"][:, nsl],
                        op0=OP.mult, op1=OP.add)
                    xb = grpp.tile([H, W], BF16, tag="xb", name="xb")
                    ve.tensor_copy(xb[:], x[:])
                    sqx = grpp.tile([H, W], BF16, tag="sqx", name="sqx")
                    alt.tensor_tensor(sqx[:], x[:], x[:], op=OP.mult)
                    ctx["x"], ctx["xb"], ctx["sqx"] = x, xb, sqx

                ln1, (l1a, l1b, l1c) = None, (None, None, None)
                ln1_parts = {}

                def p3():
                    c, parts = ln_block(lambda: ctx["x"][:],
                                        lambda: ctx["xb"][:],
                                        lambda: ctx["sqx"][:],
                                        cc["g1row_s"], 2, ln1bz, "a", W, ptile, ve)
                    ln1_parts["ctx"] = c
                    parts[0]()
                    ln1_parts["rest"] = parts[1:]

                def p4():
                    ln1_parts["rest"][0]()   # mean/rstd small ops + sqrt

                def p5():
                    ln1_parts["rest"][1]()   # pmb/prb broadcast matmuls

                def p6():  # ln1 finish -> xln (bf16)
                    c = ln1_parts["ctx"]
                    t1 = grpp.tile([H, W], F32, tag="t1", name="t1")
                    ve.tensor_tensor(t1[:], ctx["x"][:], c["pmb"][:],
                                     op=OP.subtract)
                    xln = grpp.tile([H, W], BF16, tag="xln", name="xln")
                    ve.tensor_tensor(xln[:], t1[:], c["prb"][:], op=OP.mult)
                    if not ln1bz:
                        ve.tensor_scalar(xln[:], xln[:], scalar1=col(2),
                                         scalar2=None, op0=OP.add)
                    ctx["xln"] = xln

                def p7():  # FFN half 1 (PE + ACT)
                    gf = grpp.tile([H, 4, W], BF16, tag="gf", name="gf")
                    ctx["gf"] = gf
                    if W <= 128:
                        pf = ptile([H, 4, W], "pf1")
                        for j in range(4):
                            nc.tensor.matmul(pf[:, j, :],
                                             cc["win_s"][:, 128 * j:128 * (j + 1)],
                                             ctx["xln"][:])
                        if winbz:
                            nc.scalar.activation(gf[:], pf[:], AF.Gelu, bias=0.0)
                        else:
                            for j in range(4):
                                nc.scalar.activation(gf[:, j, :], pf[:, j, :],
                                                     AF.Gelu, bias=col(4 + j))
                        return
                    pf = ptile([H, 2, W], "pf1")
                    for j in range(2):
                        nc.tensor.matmul(pf[:, j, :], cc["win_s"][:, 128 * j:128 * (j + 1)],
                                         ctx["xln"][:])
                    if winbz:
                        nc.scalar.activation(gf[:, 0:2, :], pf[:], AF.Gelu, bias=0.0)
                    else:
                        for j in range(2):
                            nc.scalar.activation(gf[:, j, :], pf[:, j, :], AF.Gelu,
                                                 bias=col(4 + j))

                def p8():  # FFN half 2
                    if W <= 128:
                        return
                    gf = ctx["gf"]
                    pf = ptile([H, 2, W], "pf2")
                    for j in range(2):
                        nc.tensor.matmul(pf[:, j, :],
                                         cc["win_s"][:, 128 * (2 + j):128 * (3 + j)],
                                         ctx["xln"][:])
                    if winbz:
                        nc.scalar.activation(gf[:, 2:4, :], pf[:], AF.Gelu, bias=0.0)
                    else:
                        for j in range(2):
                            nc.scalar.activation(gf[:, 2 + j, :], pf[:, j, :], AF.Gelu,
                                                 bias=col(6 + j))

                def p9():  # Wout + residual -> z, bf16/sq copies for LN2
                    py = ptile([H, W], "py")
                    for j in range(4):
                        nc.tensor.matmul(py[:], cc["wout_s"][:, 128 * j:128 * (j + 1)],
                                         ctx["gf"][:, j, :], start=(j == 0),
                                         stop=(woutbz and j == 3))
                    if not woutbz:
                        nc.tensor.matmul(py[:], cc["woutb_s"][:],
                                         cc["onesg_s"][:, :W],
                                         start=False, stop=True)
                    z = grpp.tile([H, W], F32, tag="z", name="z")
                    ve.tensor_tensor(z[:], ctx["xln"][:], py[:], op=OP.add)
                    zb = grpp.tile([H, W], BF16, tag="zb", name="zb")
                    ve.tensor_copy(zb[:], z[:])
                    sqz = grpp.tile([H, W], BF16, tag="sqz", name="sqz")
                    alt.tensor_tensor(sqz[:], z[:], z[:], op=OP.mult)
                    ctx["z"], ctx["zb"], ctx["sqz"] = z, zb, sqz

                ln2_parts = {}

                def p10():
                    c, parts = ln_block(lambda: ctx["zb"][:],
                                        lambda: ctx["zb"][:],
                                        lambda: ctx["sqz"][:],
                                        cc["g2row_s"], 3, ln2bz, "b", W, ptile, ve)
                    ln2_parts["ctx"] = c
                    parts[0]()
                    ln2_parts["rest"] = parts[1:]

                def p11():
                    ln2_parts["rest"][0]()
                    ln2_parts["rest"][1]()

                def p12():  # ln2 finish + (mask_V) + store
                    c = ln2_parts["ctx"]
                    t1b = grpp.tile([H, W], F32, tag="t1b", name="t1b")
                    ve.tensor_tensor(t1b[:], ctx["z"][:], c["pmb"][:],
                                     op=OP.subtract)
                    outf = grpp.tile([H, W], F32, tag="outf", name="outf")
                    ve.tensor_tensor(outf[:], t1b[:], c["prb"][:], op=OP.mult)
                    if not ln2bz:
                        ve.tensor_scalar(outf[:], outf[:], scalar1=col(3),
                                         scalar2=None, op0=OP.add)
                    if not ones_masks:
                        pmv = pgr.tile([H, W], F32, tag="gp", name="pmv")
                        nc.tensor.matmul(pmv[:], onesr_s[:], gs["mV"][:, nsl])
                        outm = grpp.tile([H, W], F32, tag="outm", name="outm")
                        ve.tensor_tensor(outm[:], outf[:], pmv[:], op=OP.mult)
                        outf = outm
                    nc.sync.dma_start(outt[:, n0:n0 + W], outf[:])
                    if last_piece:
                        gstate.pop(g)

                return [p1, p2, p3, p4, p5, p6, p7, p8, p9, p10, p11, p12]

            # ---------------- software-pipelined emission ----------------
            pending = []
            total = n_groups * n_st
            group_loads(0)
            mrow_load(0)
            mrow_load(1)
            cc.update(cold_consts())
            for t in range(total + 1):
                g, s = divmod(t, n_st)
                if t < total:
                    if s == 2 and g + 1 < n_groups:
                        group_loads(g + 1)
                    mrow_load(t + 2)
                    stage_A(t)
                if t >= 1:
                    stage_C(t - 1)
                    stage_D(t - 1)
                if t < total:
                    stage_B(t)
                if t >= 1 and s == 0 and 1 <= g < n_groups:
                    pending.extend(tail_parts(g - 1))
                # last group: staggered quarter tails, each enqueued right
                # after the D() covering its nodes, so the final dependency
                # chains overlap instead of dangling serially at the end
                if t == total - n_st // 2:
                    # last group, first half: ready right after D() of its
                    # 4th supertile; overlaps the remaining edge supertiles
                    pending.extend(tail_parts(n_groups - 1, 0, GROUP // 2,
                                              last_piece=False))
                if t == total:
                    pending.extend(tail_parts(n_groups - 1, GROUP // 2,
                                              GROUP // 2))
                npop = len(pending) if t >= total - n_st // 2 else 4
                for _ in range(npop):
                    if pending:
                        pending.pop(0)()
            while pending:
                pending.pop(0)()

    return nc


def _prep_core_inputs(h_V, h_E, mask_V, mask_attend, n_pad):
    """Host marshalling for one core's node slice (feature-major layouts)."""
    n = h_V.shape[0]
    hE8 = np.zeros((C, n_pad * K), E4)
    hE8[:, : n * K] = _f8(h_E.reshape(n * K, C)).T
    hv8 = np.zeros((H, n_pad), E4)
    hv8[:, :n] = _f8(h_V).T
    hVt = np.zeros((H, n_pad), np.float32)
    hVt[:, :n] = h_V.T
    maskr = np.zeros((1, n_pad * K), ml_dtypes.bfloat16)
    maskr[:, : n * K] = _bf(mask_attend.reshape(1, n * K))
    msum = np.zeros((1, n_pad), ml_dtypes.bfloat16)
    msum[:, :n] = _bf(mask_attend.sum(axis=1, dtype=np.float32)).reshape(1, n)
    maskVr = np.zeros((1, n_pad), ml_dtypes.bfloat16)
    maskVr[:, :n] = _bf(mask_V).reshape(1, n)
    return {
        "hE8": np.ascontiguousarray(hE8),
        "hv8D": np.ascontiguousarray(hv8),
        "hVt": np.ascontiguousarray(hVt),
        "maskr": maskr,
        "msumb": msum,
        "maskVr": maskVr,
    }


def kernel(h_V, h_E, mask_V, mask_attend,
           W1_w, W1_b, W2_w, W2_b, W3_w, W3_b,
           ln1_g, ln1_b, Win_w, Win_b, Wout_w, Wout_b, ln2_g, ln2_b):
    from concourse.bass_utils import run_bass_kernel_spmd

    _install_bir_fix()

    h_V = np.asarray(h_V, np.float32)
    h_E = np.asarray(h_E, np.float32)
    mask_V = np.asarray(mask_V, np.float32)
    mask_attend = np.asarray(mask_attend, np.float32)

    n_full = h_V.shape[0]
    per = (n_full + N_CORES - 1) // N_CORES          # 1250
    n_pad = ((per + GROUP - 1) // GROUP) * GROUP     # 1280

    w = dict(W1_w=W1_w, W1_b=W1_b, W2_w=W2_w, W2_b=W2_b, W3_w=W3_w, W3_b=W3_b,
             ln1_g=ln1_g, ln1_b=ln1_b, Win_w=Win_w, Win_b=Win_b,
             Wout_w=Wout_w, Wout_b=Wout_b, ln2_g=ln2_g, ln2_b=ln2_b)
    w = {k: np.asarray(v, np.float32) for k, v in w.items()}

    ones_masks = bool(np.all(mask_attend == 1.0) and np.all(mask_V == 1.0))
    nc = build_nc(w, n_pad, ones_masks=ones_masks)

    in_maps = []
    for c in range(N_CORES):
        lo, hi = c * per, min((c + 1) * per, n_full)
        in_maps.append(_prep_core_inputs(
            h_V[lo:hi], h_E[lo:hi], mask_V[lo:hi], mask_attend[lo:hi], n_pad
        ))

    res = run_bass_kernel_spmd(nc, in_maps, core_ids=list(range(N_CORES)))

    out = np.empty((n_full, H), np.float32)
    for c in range(N_CORES):
        lo, hi = c * per, min((c + 1) * per, n_full)
        out[lo:hi] = res.results[c]["outt"].T[: hi - lo]
    return out


# revision 64
# speedup vs baseline: 1.5120x; 1.0031x over previous
"""Trainium2 Bass kernel for nn_DecLayer (GNN message-passing decoder layer).

Strategy
--------
Data-parallel over nodes: 10000 nodes are padded and split 1280 per core
across 8 NeuronCores.  Per core everything is computed in "transposed space"
(feature dim on SBUF partitions, edges/nodes on the free dim).

Key optimizations over the v1 kernel:
  * h_E is pre-quantized to fp8(e4m3) on the host -> HBM traffic for the
    dominant stream drops 4x vs f32 (15.7MB/core, ~44us at modeled BW).
  * All edge-level matmuls run in fp8 DoubleRow perf mode (2 contraction
    tiles per pass, 0.5 cycles/row).  Weights are pre-scaled by powers of
    two into the fp8 normal range; the activation instruction's `scale`
    operand undoes the scaling for free.  Odd contraction tiles are padded
    by reading a data tile twice against zero weights (stride-0 AP).
  * The K=32 neighbor sum is folded into the W3 matmul: 32 accumulating
    PSUM matmuls with k-strided moving APs replace the (slow, no-perf-mode)
    DVE TensorReduce entirely.
  * LayerNorm stats/broadcasts use bf16 ones-matmuls (1 cycle/row instead
    of 4 for f32).
  * gelu instructions cover 1024 edges (2 PSUM banks) to amortize ACT
    SBUF-access overhead; ScalarE is the bottleneck engine at ~93us busy.
    Individual matmuls stay within one PSUM bank (512 f32) as the ISA
    requires; the 2-bank activation reads span both halves.
  * Group tails (W3/LN/FFN/LN2/store) are split into parts and interleaved
    into the supertile pipeline; the last group's tail runs as two
    half-width chains, the first overlapping the final edge supertiles.
  * h_E arrives via one merged 3-feature-chunk SWDGE descriptor per
    half-group (group 0 in progressively larger chunks so compute starts
    ~5us after launch).

Modeled per-core time: ~119us (vs 180us for the f32/bf16 v1 kernel);
measured end-to-end hardware rel-err vs the f32 reference: 5.8e-3.

If the runtime masks are all ones (true for this problem's setup_inputs),
the kernel compiles without any mask handling; a general path handles
arbitrary masks.
"""

import json

import numpy as np
import ml_dtypes

import concourse.bass as bass
import concourse.mybir as mybir
import concourse.tile as tile

F32 = mybir.dt.float32
BF16 = mybir.dt.bfloat16
FP8 = mybir.dt.float8e4
AF = mybir.ActivationFunctionType
OP = mybir.AluOpType
AX = mybir.AxisListType
PM = mybir.MatmulPerfMode

E4 = ml_dtypes.float8_e4m3

H = 128        # hidden
C = 384        # edge feature dim
K = 32         # neighbors
N_CORES = 8
GROUP = 256                      # nodes per group
ST_NODES = 32                    # nodes per supertile
ST_E = ST_NODES * K              # 1024 edges per supertile
SCALE = 30.0
EPS = 1e-5
S1 = 16.0                        # W1 fp8 pre-scale
S2 = 8.0                         # W2 fp8 pre-scale
S3 = 256.0 / SCALE               # W3 fp8 pre-scale (folds 1/SCALE)


# ---------------------------------------------------------------------------
# walrus workaround: this build rejects >1 sync wait per instruction; split
# extra waits into standalone EventSemaphore instructions on the same engine
# (engines execute their stream in order, so semantics are preserved).
# ---------------------------------------------------------------------------
def _split_multi_waits(bir_json: bytes) -> bytes:
    m = json.loads(bir_json)
    for f in m.get("functions", []):
        for b in f.get("blocks", []):
            out = []
            for inst in b.get("instructions", []):
                si = inst.get("sync_info")
                waits = (si or {}).get("on_wait") or []
                if len(waits) > 1:
                    for j, w in enumerate(waits[:-1]):
                        out.append({
                            "debug": inst.get("debug", 0),
                            "engine": inst["engine"],
                            "ins": [], "outs": [],
                            "name": f"{inst['name']}_wsplit{j}",
                            "opcode": "EventSemaphore",
                            "sync_info": {"on_update": [], "on_wait": [w]},
                        })
                    si["on_wait"] = waits[-1:]
                out.append(inst)
            b["instructions"] = out
    return json.dumps(m).encode()


def _install_bir_fix():
    import concourse.bass_utils as bu
    import concourse.bass2jax as b2j
    if getattr(bu, "_wsplit_installed", False):
        return
    orig = bu.compile_bir_kernel

    def patched(bir_json, tmpdir, neff_name="file.neff"):
        return orig(_split_multi_waits(bir_json), tmpdir, neff_name)

    bu.compile_bir_kernel = patched
    b2j.compile_bir_kernel = patched
    bu._wsplit_installed = True


def _bf(x):
    return np.asarray(x, dtype=ml_dtypes.bfloat16)


def _f8(x):
    return np.asarray(x, dtype=E4)


def _dr_pair(a, b):
    """Stack two [128, M] weight blocks into DoubleRow lhsT layout
    [128, 2*M] = [p, (t m)]."""
    return np.stack([a, b], axis=1).reshape(128, -1)


def build_nc(w, n_nodes, ones_masks=False):
    """Build the per-core Bass kernel. `w` holds the (host numpy) weights."""
    assert n_nodes % GROUP == 0
    n_groups = n_nodes // GROUP
    n_edges = n_nodes * K
    n_st = GROUP // ST_NODES

    nc = bass.Bass()

    hE8 = nc.dram_tensor("hE8", [C, n_edges], FP8, kind="ExternalInput")
    hv8D = nc.dram_tensor("hv8D", [H, n_nodes], FP8, kind="ExternalInput")
    hVt = nc.dram_tensor("hVt", [H, n_nodes], F32, kind="ExternalInput")
    maskr = nc.dram_tensor("maskr", [1, n_edges], BF16, kind="ExternalInput")
    msumb = nc.dram_tensor("msumb", [1, n_nodes], BF16, kind="ExternalInput")
    maskVr = nc.dram_tensor("maskVr", [1, n_nodes], BF16, kind="ExternalInput")
    outt = nc.dram_tensor("outt", [H, n_nodes], F32, kind="ExternalOutput")

    # --- inline constants (weights known at build time) ---
    W1 = w["W1_w"]          # [H+C, H]
    zero8 = np.zeros((128, H), E4)
    w1a_h = _dr_pair(_f8(W1[H:H + 128] * S1), _f8(W1[H + 128:H + 256] * S1))
    w1b_h = _dr_pair(_f8(W1[H + 256:H + 384] * S1), zero8)
    w1v_h = _dr_pair(_f8(W1[:H] * S1), zero8)
    w2d_h = _dr_pair(_f8(w["W2_w"] * S2), zero8)
    w3d_h = _dr_pair(_f8(w["W3_w"] * S3), zero8)
    w3b_h = _bf(w["W3_w"] / SCALE)                      # general-mask path
    wout_h = np.concatenate(
        [w["Wout_w"][128 * j: 128 * (j + 1), :] for j in range(4)], axis=1)

    w1aD = nc.inline_tensor(w1a_h, name="w1a")
    w1bD = nc.inline_tensor(w1b_h, name="w1b")
    w1vD = nc.inline_tensor(w1v_h, name="w1v")
    w2dD = nc.inline_tensor(w2d_h, name="w2d")
    if ones_masks:
        w3dD = nc.inline_tensor(w3d_h, name="w3d")
    else:
        w3bD = nc.inline_tensor(w3b_h, name="w3b")
        b3sD = nc.inline_tensor(_bf(w["W3_b"] / SCALE).reshape(1, H), name="b3s")
    winD = nc.inline_tensor(_bf(w["Win_w"]), name="win")
    woutD = nc.inline_tensor(_bf(wout_h), name="wout")
    woutbD = nc.inline_tensor(_bf(w["Wout_b"]).reshape(1, H), name="woutb")
    onescD = nc.inline_tensor(np.ones((H, 1), ml_dtypes.bfloat16), name="onesc")
    onesrD = nc.inline_tensor(np.ones((1, H), ml_dtypes.bfloat16), name="onesr")
    onesgD = nc.inline_tensor(np.ones((1, GROUP), ml_dtypes.bfloat16), name="onesg")
    g1rowD = nc.inline_tensor(_bf(w["ln1_g"]).reshape(1, H), name="g1row")
    g2rowD = nc.inline_tensor(_bf(w["ln2_g"]).reshape(1, H), name="g2row")
    # per-partition bias columns: b1, b2, ln1b, ln2b, winb0..3
    cols_h = np.stack(
        [w["W1_b"], w["W2_b"], w["ln1_b"], w["ln2_b"]]
        + [w["Win_b"][128 * j: 128 * (j + 1)] for j in range(4)]
        + [np.full(H, EPS, np.float32)],
        axis=1,
    ).astype(np.float32)
    colsD = nc.inline_tensor(cols_h, name="cols")

    # value specializations (checked against actual weights at build time)
    b1z = not np.any(w["W1_b"])
    b2z = not np.any(w["W2_b"])
    b3z = not np.any(w["W3_b"])
    winbz = not np.any(w["Win_b"])
    woutbz = not np.any(w["Wout_b"])
    ln1bz = not np.any(w["ln1_b"])
    ln2bz = not np.any(w["ln2_b"])

    g2_dt = FP8 if ones_masks else BF16

    with tile.TileContext(nc) as tc:
        with (
            tc.tile_pool(name="const", bufs=1) as constp,
            tc.tile_pool(name="xe", bufs=3) as xep,
            tc.tile_pool(name="g1p", bufs=3) as g1p,
            tc.tile_pool(name="g2p", bufs=3) as g2p,
            tc.tile_pool(name="st", bufs=3) as stp,
            tc.tile_pool(name="grp", bufs=3) as grpp,
            tc.tile_pool(name="ph", bufs=3 if ones_masks else 2,
                         space="PSUM") as php,
            tc.tile_pool(name="pm", bufs=1, space="PSUM") as pmp,
            tc.tile_pool(name="gp", bufs=2, space="PSUM") as pgr,
        ):
            # ---- load constants once (SP/HWDGE; bulk streams go via Pool) ----
            def cload(shape, dt_, src, name):
                t = constp.tile(shape, dt_, name=name)
                nc.sync.dma_start(t[:], src)
                return t

            # hot constants: needed by the first supertile's W1/gelu1/W2
            w1a_s = cload([128, 2, H], FP8, w1aD[:].rearrange("p (t m) -> p t m", t=2), "w1a_s")
            w1b_s = cload([128, 2, H], FP8, w1bD[:].rearrange("p (t m) -> p t m", t=2), "w1b_s")
            w1v_s = cload([128, 2, H], FP8, w1vD[:].rearrange("p (t m) -> p t m", t=2), "w1v_s")
            onesr_s = cload([1, H], BF16, onesrD[:], "onesr_s")

            def cold_consts():
                """Emitted after group-0 loads: first used by W2 / the tails."""
                c = {}
                c["w2d_s"] = cload([128, 2, H], FP8,
                                   w2dD[:].rearrange("p (t m) -> p t m", t=2), "w2d_s")
                c["cols_s"] = cload([H, cols_h.shape[1]], F32, colsD[:], "cols_s")
                if ones_masks:
                    c["w3d_s"] = cload([128, 2, H], FP8,
                                       w3dD[:].rearrange("p (t m) -> p t m", t=2), "w3d_s")
                else:
                    c["w3b_s"] = cload([128, H], BF16, w3bD[:], "w3b_s")
                    c["b3s_s"] = cload([1, H], BF16, b3sD[:], "b3s_s")
                c["win_s"] = cload([H, 512], BF16, winD[:], "win_s")
                c["wout_s"] = cload([H, 512], BF16, woutD[:], "wout_s")
                c["woutb_s"] = cload([1, H], BF16, woutbD[:], "woutb_s")
                c["onesc_s"] = cload([H, 1], BF16, onescD[:], "onesc_s")
                c["onesg_s"] = cload([1, GROUP], BF16, onesgD[:], "onesg_s")
                c["g1row_s"] = cload([1, H], BF16, g1rowD[:], "g1row_s")
                c["g2row_s"] = cload([1, H], BF16, g2rowD[:], "g2row_s")
                return c

            cc = {}

            def col(i):
                return cc["cols_s"][:, i:i + 1]

            # ---------------- per-group state ----------------
            gstate = {}   # g -> dict of tiles
            ststate = {}  # t -> dict of tiles
            mstate = {}

            def group_loads(g):
                e0 = g * GROUP * K
                n0 = g * GROUP
                st = {}
                st["hv8"] = grpp.tile([H, GROUP], FP8, tag="hv8", name="hv8")
                # group 0's hv8 gates the very first W1 matmul: route it via
                # the Pool SWDGE queue, which starts draining before the
                # HWDGE const queue reaches it
                heng = nc.gpsimd if g == 0 else nc.sync
                heng.dma_start(st["hv8"][:], hv8D[:, n0:n0 + GROUP])
                st["hvf"] = grpp.tile([H, GROUP], F32, tag="hvf", name="hvf")
                nc.sync.dma_start(st["hvf"][:], hVt[:, n0:n0 + GROUP])
                if not ones_masks:
                    st["mV"] = grpp.tile([1, GROUP], BF16, tag="mV", name="mV")
                    nc.sync.dma_start(st["mV"][:], maskVr[:, n0:n0 + GROUP])
                    if not b3z:
                        st["msum"] = grpp.tile([1, GROUP], BF16, tag="msum", name="msum")
                        nc.sync.dma_start(st["msum"][:], msumb[:, n0:n0 + GROUP])
                # bulk h_E stream: chunked via gpsimd SWDGE so the HWDGE/SP
                # queue stays free for small loads and stores.  Group 0 uses
                # progressively larger chunks so the first W1 starts early.
                st["xe"] = xep.tile([H, 3, GROUP * K], FP8, tag="xe", name="xe")
                hEr = hE8[:].rearrange("(j p) e -> p j e", p=128)
                if g == 0:
                    bounds = [0, 1024, 2048, 3072, 4096, 6144, 8192]
                else:
                    bounds = [0, 2048, 4096, 8192]
                for hh in range(len(bounds) - 1):
                    nc.gpsimd.dma_start(
                        st["xe"][:, :, bounds[hh]:bounds[hh + 1]],
                        hEr[:, :, e0 + bounds[hh]:e0 + bounds[hh + 1]],
                    )
                st["g2"] = g2p.tile([H, GROUP * K], g2_dt, tag="g2", name="g2")
                gstate[g] = st

            def mrow_load(t):
                if ones_masks or t >= n_groups * n_st:
                    return
                mrow = stp.tile([1, ST_E], BF16, tag="mrow", name="mrow")
                nc.sync.dma_start(mrow[:], maskr[:, t * ST_E:(t + 1) * ST_E])
                mstate[t] = mrow

            def stage_A(t):
                """W1 matmuls for supertile t -> ph (2 PSUM banks)."""
                g, s = divmod(t, n_st)
                gs = gstate[g]
                c0 = s * ST_E
                st = {}
                ph1 = php.tile([H, ST_E], F32, tag="ph", name="ph1")
                HB = ST_E // 2          # matmul out must stay in one PSUM bank
                HN = ST_NODES // 2
                for h in range(2):
                    dst = ph1[:, h * HB:(h + 1) * HB]
                    e0h = c0 + h * HB
                    nc.tensor.matmul(dst, w1a_s[:], gs["xe"][:, 0:2, e0h:e0h + HB],
                                     start=True, stop=False, perf_mode=PM.DoubleRow)
                    nc.tensor.matmul(
                        dst, w1b_s[:],
                        gs["xe"][:, 2:3, e0h:e0h + HB].broadcast_to([H, 2, HB]),
                        start=False, stop=False, perf_mode=PM.DoubleRow)
                    n0h = s * ST_NODES + h * HN
                    hv_b = (gs["hv8"][:, n0h:n0h + HN]
                            .rearrange("p (o n) -> p o n", o=1)
                            .broadcast_to([H, 2, HN])
                            .broadcast_to([H, 2, HN, K]))
                    nc.tensor.matmul(dst, w1v_s[:], hv_b,
                                     start=False, stop=True, perf_mode=PM.DoubleRow)
                st["ph1"] = ph1
                if not ones_masks:
                    pm = pmp.tile([H, ST_E], F32, tag="pm", name="pm")
                    mrow = mstate.pop(t)
                    for h in range(2):
                        nc.tensor.matmul(pm[:, h * (ST_E // 2):(h + 1) * (ST_E // 2)],
                                         onesr_s[:],
                                         mrow[:, h * (ST_E // 2):(h + 1) * (ST_E // 2)])
                    st["pm"] = pm
                ststate[t] = st

            def stage_B(t):
                """gelu1 over 1024 edges; un-scales W1 by 1/S1; fp8 out."""
                st = ststate[t]
                g1 = g1p.tile([H, 1, ST_E], FP8, tag="g1", name="g1")
                nc.scalar.activation(g1[:, 0, :], st["ph1"][:], AF.Gelu,
                                     bias=0.0 if b1z else col(0), scale=1.0 / S1)
                st["g1"] = g1

            def stage_C(t):
                """W2 fp8 DoubleRow (dup-tile) -> ph2."""
                st = ststate[t]
                ph2 = php.tile([H, ST_E], F32, tag="ph", name="ph2")
                HB = ST_E // 2
                for h in range(2):
                    nc.tensor.matmul(
                        ph2[:, h * HB:(h + 1) * HB], cc["w2d_s"][:],
                        st["g1"][:, 0:1, h * HB:(h + 1) * HB]
                        .broadcast_to([H, 2, HB]),
                        start=True, stop=True, perf_mode=PM.DoubleRow)
                st["ph2"] = ph2

            def stage_D(t):
                """gelu2 -> slice of the group g2 tile (fp8 or bf16)."""
                g, s = divmod(t, n_st)
                st = ststate.pop(t)
                gs = gstate[g]
                dst = gs["g2"][:, s * ST_E:(s + 1) * ST_E]
                if ones_masks:
                    nc.scalar.activation(dst, st["ph2"][:], AF.Gelu,
                                         bias=0.0 if b2z else col(1), scale=1.0 / S2)
                else:
                    g2t = stp.tile([H, ST_E], BF16, tag="g2t", name="g2t")
                    nc.scalar.activation(g2t[:], st["ph2"][:], AF.Gelu,
                                         bias=0.0 if b2z else col(1), scale=1.0 / S2)
                    nc.vector.tensor_tensor(dst, g2t[:], st["pm"][:], op=OP.mult)

            # ---------------- group tails ----------------
            def ln_block(src_f32, src_bf, sq_bf, gamma_row, beta_i, beta_z, nm, W,
                         ptile, ve):
                """Shared LN piece: stats matmuls -> mean/rstd -> normalized.

                src_f32: [H, GROUP] f32 (the residual input)
                src_bf/sq_bf: bf16 copies (src, src^2) for the stats matmuls
                Returns list of part-functions; leaves result tile in ctx[nm].
                """
                ctx = {}

                def s1():
                    ps1 = pgr.tile([1, W], F32, tag="gp", name=f"ps1{nm}")
                    nc.tensor.matmul(ps1[:], cc["onesc_s"][:], src_bf())
                    ps2 = pgr.tile([1, W], F32, tag="gp", name=f"ps2{nm}")
                    nc.tensor.matmul(ps2[:], cc["onesc_s"][:], sq_bf())
                    ctx["ps1"], ctx["ps2"] = ps1, ps2

                def s2():
                    mean = grpp.tile([1, W], BF16, tag=f"mean{nm}", name=f"mean{nm}")
                    tmp = grpp.tile([1, W], F32, tag=f"m2{nm}", name=f"m2{nm}")
                    varh = grpp.tile([1, W], F32, tag=f"var{nm}", name=f"var{nm}")
                    sd = grpp.tile([1, W], F32, tag=f"sd{nm}", name=f"sd{nm}")
                    rstd = grpp.tile([1, W], BF16, tag=f"rstd{nm}", name=f"rstd{nm}")
                    with nc.allow_low_precision("bf16 LN stats validated vs ref"):
                        ve.tensor_scalar_mul(mean[:], ctx["ps1"][:], 1.0 / H)
                        ve.tensor_tensor(tmp[:], mean[:], mean[:], op=OP.mult)
                        ve.scalar_tensor_tensor(
                            varh[:], ctx["ps2"][:], 1.0 / H, tmp[:],
                            op0=OP.mult, op1=OP.subtract)
                        nc.scalar.activation(sd[:], varh[:], AF.Sqrt,
                                             bias=cc["cols_s"][0:1, 8:9])
                        nc.vector.reciprocal(rstd[:], sd[:])
                    ctx["mean"], ctx["rstd"] = mean, rstd

                def s3():
                    pmb = ptile([H, W], f"pmb{nm}")
                    nc.tensor.matmul(pmb[:], onesr_s[:], ctx["mean"][:])
                    prb = ptile([H, W], f"prb{nm}")
                    nc.tensor.matmul(prb[:], gamma_row[:], ctx["rstd"][:])
                    ctx["pmb"], ctx["prb"] = pmb, prb

                return ctx, [s1, s2, s3]

            def tail_parts(g, lo=0, W=GROUP, last_piece=True, out=None):
                gs = gstate[g]
                n0 = g * GROUP + lo
                nsl = slice(lo, lo + W)
                fin = (g == n_groups - 1) and lo > 0
                ve = nc.vector
                alt = nc.gpsimd
                def ptile(shape, name):
                    if fin:
                        return php.tile(shape, F32, tag="ph", name=name)
                    return pgr.tile(shape, F32, tag="gp", name=name)
                ctx = {}

                def p1(kr=None):  # K-sum folded into W3 (PE)
                    if kr is None:
                        kr = range(K)
                    if "pdh" not in ctx:
                        ctx["pdh"] = pgr.tile([H, W], F32, tag="gp", name="pdh")
                        ctx["g2k"] = (gs["g2"][:]
                                      .rearrange("p (n k) -> p k n", k=K)[:, :, nsl])
                    pdh, g2k = ctx["pdh"], ctx["g2k"]
                    if ones_masks:
                        for k in kr:
                            nc.tensor.matmul(
                                pdh[:], cc["w3d_s"][:],
                                g2k[:, k:k + 1, :].broadcast_to([H, 2, W]),
                                start=(k == 0), stop=(k == K - 1),
                                perf_mode=PM.DoubleRow)
                    else:
                        last = b3z
                        for k in kr:
                            nc.tensor.matmul(
                                pdh[:], cc["w3b_s"][:], g2k[:, k, :],
                                start=(k == 0), stop=(last and k == K - 1))
                        if kr[-1] == K - 1 and not b3z:
                            nc.tensor.matmul(pdh[:], cc["b3s_s"][:],
                                             gs["msum"][:, nsl],
                                             start=False, stop=True)
                    ctx["pdh"] = pdh

                def p2():  # residual add + bf16/sq copies for LN1 stats
                    x = grpp.tile([H, W], F32, tag="x", name="x")
                    pscale = 1.0 / 256.0 if ones_masks else 1.0
                    ve.scalar_tensor_tensor(
                        x[:], ctx["pdh"][:], pscale, gs["hvf"][:, nsl],
                        op0=OP.mult, op1=OP.add)
                    xb = grpp.tile([H, W], BF16, tag="xb", name="xb")
                    ve.tensor_copy(xb[:], x[:])
                    sqx = grpp.tile([H, W], BF16, tag="sqx", name="sqx")
                    alt.tensor_tensor(sqx[:], x[:], x[:], op=OP.mult)
                    ctx["x"], ctx["xb"], ctx["sqx"] = x, xb, sqx

                ln1, (l1a, l1b, l1c) = None, (None, None, None)
                ln1_parts = {}

                def p3():
                    c, parts = ln_block(lambda: ctx["x"][:],
                                        lambda: ctx["xb"][:],
                                        lambda: ctx["sqx"][:],
                                        cc["g1row_s"], 2, ln1bz, "a", W, ptile, ve)
                    ln1_parts["ctx"] = c
                    parts[0]()
                    ln1_parts["rest"] = parts[1:]

                def p4():
                    ln1_parts["rest"][0]()   # mean/rstd small ops + sqrt

                def p5():
                    ln1_parts["rest"][1]()   # pmb/prb broadcast matmuls

                def p6():  # ln1 finish -> xln (bf16)
                    c = ln1_parts["ctx"]
                    t1 = grpp.tile([H, W], F32, tag="t1", name="t1")
                    ve.tensor_tensor(t1[:], ctx["x"][:], c["pmb"][:],
                                     op=OP.subtract)
                    xln = grpp.tile([H, W], BF16, tag="xln", name="xln")
                    ve.tensor_tensor(xln[:], t1[:], c["prb"][:], op=OP.mult)
                    if not ln1bz:
                        ve.tensor_scalar(xln[:], xln[:], scalar1=col(2),
                                         scalar2=None, op0=OP.add)
                    ctx["xln"] = xln

                def p7():  # FFN half 1 (PE + ACT)
                    gf = grpp.tile([H, 4, W], BF16, tag="gf", name="gf")
                    ctx["gf"] = gf
                    if W <= 128:
                        pf = ptile([H, 4, W], "pf1")
                        for j in range(4):
                            nc.tensor.matmul(pf[:, j, :],
                                             cc["win_s"][:, 128 * j:128 * (j + 1)],
                                             ctx["xln"][:])
                        if winbz:
                            nc.scalar.activation(gf[:], pf[:], AF.Gelu, bias=0.0)
                        else:
                            for j in range(4):
                                nc.scalar.activation(gf[:, j, :], pf[:, j, :],
                                                     AF.Gelu, bias=col(4 + j))
                        return
                    pf = ptile([H, 2, W], "pf1")
                    for j in range(2):
                        nc.tensor.matmul(pf[:, j, :], cc["win_s"][:, 128 * j:128 * (j + 1)],
                                         ctx["xln"][:])
                    if winbz:
                        nc.scalar.activation(gf[:, 0:2, :], pf[:], AF.Gelu, bias=0.0)
                    else:
                        for j in range(2):
                            nc.scalar.activation(gf[:, j, :], pf[:, j, :], AF.Gelu,
                                                 bias=col(4 + j))

                def p8():  # FFN half 2
                    if W <= 128:
                        return
                    gf = ctx["gf"]
                    pf = ptile([H, 2, W], "pf2")
                    for j in range(2):
                        nc.tensor.matmul(pf[:, j, :],
                                         cc["win_s"][:, 128 * (2 + j):128 * (3 + j)],
                                         ctx["xln"][:])
                    if winbz:
                        nc.scalar.activation(gf[:, 2:4, :], pf[:], AF.Gelu, bias=0.0)
                    else:
                        for j in range(2):
                            nc.scalar.activation(gf[:, 2 + j, :], pf[:, j, :], AF.Gelu,
                                                 bias=col(6 + j))

                def p9():  # Wout + residual -> z, bf16/sq copies for LN2
                    py = ptile([H, W], "py")
                    for j in range(4):
                        nc.tensor.matmul(py[:], cc["wout_s"][:, 128 * j:128 * (j + 1)],
                                         ctx["gf"][:, j, :], start=(j == 0),
                                         stop=(woutbz and j == 3))
                    if not woutbz:
                        nc.tensor.matmul(py[:], cc["woutb_s"][:],
                                         cc["onesg_s"][:, :W],
                                         start=False, stop=True)
                    z = grpp.tile([H, W], F32, tag="z", name="z")
                    ve.tensor_tensor(z[:], ctx["xln"][:], py[:], op=OP.add)
                    zb = grpp.tile([H, W], BF16, tag="zb", name="zb")
                    ve.tensor_copy(zb[:], z[:])
                    sqz = grpp.tile([H, W], BF16, tag="sqz", name="sqz")
                    alt.tensor_tensor(sqz[:], z[:], z[:], op=OP.mult)
                    ctx["z"], ctx["zb"], ctx["sqz"] = z, zb, sqz

                ln2_parts = {}

                def p10():
                    c, parts = ln_block(lambda: ctx["zb"][:],
                                        lambda: ctx["zb"][:],
                                        lambda: ctx["sqz"][:],
                                        cc["g2row_s"], 3, ln2bz, "b", W, ptile, ve)
                    ln2_parts["ctx"] = c
                    parts[0]()
                    ln2_parts["rest"] = parts[1:]

                def p11():
                    ln2_parts["rest"][0]()
                    ln2_parts["rest"][1]()

                def p12():  # ln2 finish + (mask_V) + store
                    c = ln2_parts["ctx"]
                    t1b = grpp.tile([H, W], F32, tag="t1b", name="t1b")
                    ve.tensor_tensor(t1b[:], ctx["z"][:], c["pmb"][:],
                                     op=OP.subtract)
                    if out is not None:
                        # write into a shared output tile; one store covers
                        # all pieces sharing it (issued by the designated one)
                        ot, off, do_store, span = out
                        dst = ot[:, off:off + W]
                        ve.tensor_tensor(dst, t1b[:], c["prb"][:], op=OP.mult)
                        if not ln2bz:
                            ve.tensor_scalar(dst, dst, scalar1=col(3),
                                             scalar2=None, op0=OP.add)
                        if do_store:
                            nc.sync.dma_start(outt[:, n0 - off:n0 - off + span],
                                              ot[:])
                        if last_piece:
                            gstate.pop(g)
                        return
                    outf = grpp.tile([H, W], F32, tag="outf", name="outf")
                    ve.tensor_tensor(outf[:], t1b[:], c["prb"][:], op=OP.mult)
                    if not ln2bz:
                        ve.tensor_scalar(outf[:], outf[:], scalar1=col(3),
                                         scalar2=None, op0=OP.add)
                    if not ones_masks:
                        pmv = pgr.tile([H, W], F32, tag="gp", name="pmv")
                        nc.tensor.matmul(pmv[:], onesr_s[:], gs["mV"][:, nsl])
                        outm = grpp.tile([H, W], F32, tag="outm", name="outm")
                        ve.tensor_tensor(outm[:], outf[:], pmv[:], op=OP.mult)
                        outf = outm
                    nc.sync.dma_start(outt[:, n0:n0 + W], outf[:])
                    if last_piece:
                        gstate.pop(g)

                return [p1, p2, p3, p4, p5, p6, p7, p8, p9, p10, p11, p12]

            # ---------------- software-pipelined emission ----------------
            pending = []
            total = n_groups * n_st
            group_loads(0)
            mrow_load(0)
            mrow_load(1)
            cc.update(cold_consts())
            for t in range(total + 1):
                g, s = divmod(t, n_st)
                if t < total:
                    if s == 2 and g + 1 < n_groups:
                        group_loads(g + 1)
                    mrow_load(t + 2)
                    stage_A(t)
                if t >= 1:
                    stage_C(t - 1)
                    stage_D(t - 1)
                if t < total:
                    stage_B(t)
                if t >= 1 and s == 0 and 1 <= g < n_groups:
                    pending.extend(tail_parts(g - 1))
                # last group: staggered quarter tails, each enqueued right
                # after the D() covering its nodes, so the final dependency
                # chains overlap instead of dangling serially at the end
                if t == total - n_st // 2:
                    # last group, first half: ready right after D() of its
                    # 4th supertile; overlaps the remaining edge supertiles
                    pending.extend(tail_parts(n_groups - 1, 0, GROUP // 2,
                                              last_piece=False))
                if t == total:
                    pending.extend(tail_parts(n_groups - 1, GROUP // 2,
                                              GROUP // 2))
                npop = len(pending) if t >= total - n_st // 2 else 4
                for _ in range(npop):
                    if pending:
                        pending.pop(0)()
            while pending:
                pending.pop(0)()

    return nc


def _prep_core_inputs(h_V, h_E, mask_V, mask_attend, n_pad):
    """Host marshalling for one core's node slice (feature-major layouts)."""
    n = h_V.shape[0]
    hE8 = np.zeros((C, n_pad * K), E4)
    hE8[:, : n * K] = _f8(h_E.reshape(n * K, C)).T
    hv8 = np.zeros((H, n_pad), E4)
    hv8[:, :n] = _f8(h_V).T
    hVt = np.zeros((H, n_pad), np.float32)
    hVt[:, :n] = h_V.T
    maskr = np.zeros((1, n_pad * K), ml_dtypes.bfloat16)
    maskr[:, : n * K] = _bf(mask_attend.reshape(1, n * K))
    msum = np.zeros((1, n_pad), ml_dtypes.bfloat16)
    msum[:, :n] = _bf(mask_attend.sum(axis=1, dtype=np.float32)).reshape(1, n)
    maskVr = np.zeros((1, n_pad), ml_dtypes.bfloat16)
    maskVr[:, :n] = _bf(mask_V).reshape(1, n)
    return {
        "hE8": np.ascontiguousarray(hE8),
        "hv8D": np.ascontiguousarray(hv8),
        "hVt": np.ascontiguousarray(hVt),
        "maskr": maskr,
        "msumb": msum,
        "maskVr": maskVr,
    }


def kernel(h_V, h_E, mask_V, mask_attend,
           W1_w, W1_b, W2_w, W2_b, W3_w, W3_b,
           ln1_g, ln1_b, Win_w, Win_b, Wout_w, Wout_b, ln2_g, ln2_b):
    from concourse.bass_utils import run_bass_kernel_spmd

    _install_bir_fix()

    h_V = np.asarray(h_V, np.float32)
    h_E = np.asarray(h_E, np.float32)
    mask_V = np.asarray(mask_V, np.float32)
    mask_attend = np.asarray(mask_attend, np.float32)

    n_full = h_V.shape[0]
    per = (n_full + N_CORES - 1) // N_CORES          # 1250
    n_pad = ((per + GROUP - 1) // GROUP) * GROUP     # 1280

    w = dict(W1_w=W1_w, W1_b=W1_b, W2_w=W2_w, W2_b=W2_b, W3_w=W3_w, W3_b=W3_b,
             ln1_g=ln1_g, ln1_b=ln1_b, Win_w=Win_w, Win_b=Win_b,
             Wout_w=Wout_w, Wout_b=Wout_b, ln2_g=ln2_g, ln2_b=ln2_b)
    w = {k: np.asarray(v, np.float32) for k, v in w.items()}

    ones_masks = bool(np.all(mask_attend == 1.0) and np.all(mask_V == 1.0))
    nc = build_nc(w, n_pad, ones_masks=ones_masks)

    in_maps = []
    for c in range(N_CORES):
        lo, hi = c * per, min((c + 1) * per, n_full)
        in_maps.append(_prep_core_inputs(
            h_V[lo:hi], h_E[lo:hi], mask_V[lo:hi], mask_attend[lo:hi], n_pad
        ))

    res = run_bass_kernel_spmd(nc, in_maps, core_ids=list(range(N_CORES)))

    out = np.empty((n_full, H), np.float32)
    for c in range(N_CORES):
        lo, hi = c * per, min((c + 1) * per, n_full)
        out[lo:hi] = res.results[c]["outt"].T[: hi - lo]
    return out
